# revision 1
# baseline (speedup 1.0000x reference)
"""Trainium2 Bass kernel for nn_AggregateStgcn (gnn_message_passing).

Computes, for x:(1,16,1,8192) f32, graph:(8192,8192) f32, fifo:(1,16,4,8192) f32,
stride=2:
    A[ck, v]   = x[0, ck, 0, v]                       (16, 8192)
    Asum[k, v] = sum_c A[c*4+k, v]                    (4, 8192)
    xsum[k, w] = sum_v Asum[k, v] * graph[v, w]       (4, 8192)
    S[k, w]    = sum_{j in 1,3,...,13} fifo[0, j, k, w]
    out[0, k, w, 0] = xsum[k, w] + S[k, w]            (1, 4, 8192, 1)

Sharding: graph is split column-wise across 8 NeuronCores (tensor parallel over
output nodes w); x is replicated; the fifo slice is local per core. No
collectives; host concatenates the 8 (4, 1024) output slices.

Device kernel (per core, w-slice of 1024 columns):
  - prep: 64 small PE matmuls (lhsT = x16 column block (16,128), rhs = a
    (16,4) selection matrix) produce AsumT tiles (128,4) directly in PSUM:
    the matmul performs both the transpose (v -> partitions) and the c-sum.
  - main: for each 128-row tile of the graph slice, one accumulating matmul
    (4, 512) x2 halves. The fifo strided sum rides the same PSUM accumulation
    as one extra matmul with a (28, 4) selection matrix.
  - epilogue: PSUM -> SBUF copy, DMA out.
"""

import numpy as np

V = 8192
C = 4
K = 4
F = 16
NCORES = 8
WS = V // NCORES          # 1024 output columns per core
NT = V // 128             # 64 contraction tiles
CHUNK_T = 4               # graph v-tiles per DMA (4 * 512KB = 2MB per DMA)
NCHUNKS = NT // CHUNK_T
GBUFS = 5                 # graph chunk buffers in SBUF

TRACE = False             # set by test harness to capture an NTFF profile
LAST = None               # BassKernelResults of the most recent run

_CACHED_NC = None


def _build_nc():
    import concourse.bacc as bacc
    import concourse.mybir as mybir
    from concourse.tile import TileContext

    f32 = mybir.dt.float32
    nc = bacc.Bacc(
        "TRN2",
        target_bir_lowering=False,
        debug=False,
        enable_asserts=False,
        num_devices=NCORES,
    )
    g = nc.dram_tensor("g", [V, WS], f32, kind="ExternalInput")
    xs = nc.dram_tensor("xs", [C * K, V], f32, kind="ExternalInput")
    ff = nc.dram_tensor("ff", [7 * C, WS], f32, kind="ExternalInput")
    selr = nc.dram_tensor("selr", [C * K, K], f32, kind="ExternalInput")
    selfm = nc.dram_tensor("selfm", [7 * C, K], f32, kind="ExternalInput")
    out = nc.dram_tensor("out", [K, WS], f32, kind="ExternalOutput")

    with TileContext(nc) as tc:
        with (
            tc.tile_pool(name="const", bufs=1) as cpool,
            tc.tile_pool(name="gp", bufs=GBUFS) as gpool,
            tc.tile_pool(name="ps", bufs=1, space="PSUM") as ppool,
        ):
            x_sb = cpool.tile([C * K, V], f32)
            nc.sync.dma_start(out=x_sb[:], in_=xs.ap())
            selr_sb = cpool.tile([C * K, K], f32)
            nc.sync.dma_start(out=selr_sb[:], in_=selr.ap())
            selfm_sb = cpool.tile([7 * C, K], f32)
            nc.sync.dma_start(out=selfm_sb[:], in_=selfm.ap())
            ff_sb = cpool.tile([7 * C, WS], f32)
            nc.sync.dma_start(out=ff_sb[:], in_=ff.ap())

            # prep: AsumT (v on partitions, k free) via PE; one (16,128)x(16,4)
            # matmul per v-tile packs all 64 results into one PSUM tile.
            prep_ps = ppool.tile([128, NT * K], f32)
            for t in range(NT):
                nc.tensor.matmul(
                    prep_ps[:, t * K : (t + 1) * K],
                    x_sb[:, t * 128 : (t + 1) * 128],
                    selr_sb[:],
                    start=True,
                    stop=True,
                )
            lhsTp = cpool.tile([128, NT * K], f32)
            nc.vector.tensor_copy(out=lhsTp[:], in_=prep_ps[:])

            # accumulators, one PSUM bank per 512-wide output half;
            # the fifo matmul opens each accumulation group.
            acc = []
            for h in range(2):
                a = ppool.tile([K, 512], f32, name=f"acc{h}", tag=f"acc{h}")
                acc.append(a)
                nc.tensor.matmul(
                    a[:],
                    selfm_sb[:],
                    ff_sb[:, h * 512 : (h + 1) * 512],
                    start=True,
                    stop=False,
                )

            gv = g.ap().rearrange("(c j p) w -> c p j w", j=CHUNK_T, p=128)
            for ci in range(NCHUNKS):
                gt = gpool.tile([128, CHUNK_T, WS], f32, name="gt", tag="gt")
                nc.sync.dma_start(out=gt[:], in_=gv[ci])
                for j in range(CHUNK_T):
                    t = ci * CHUNK_T + j
                    for h in range(2):
                        nc.tensor.matmul(
                            acc[h][:],
                            lhsTp[:, t * K : (t + 1) * K],
                            gt[:, j, h * 512 : (h + 1) * 512],
                            start=False,
                            stop=(t == NT - 1),
                        )

            out_sb = cpool.tile([K, WS], f32)
            for h in range(2):
                nc.vector.tensor_copy(
                    out=out_sb[:, h * 512 : (h + 1) * 512], in_=acc[h][:]
                )
            nc.sync.dma_start(out=out.ap(), in_=out_sb[:])

    nc.compile()
    return nc


def kernel(x, graph, fifo, stride):
    global _CACHED_NC, LAST
    from concourse.bass_utils import run_bass_kernel_spmd

    x = np.asarray(x, dtype=np.float32)
    graph = np.asarray(graph, dtype=np.float32)
    fifo = np.asarray(fifo, dtype=np.float32)
    stride_v = int(np.asarray(stride))
    assert stride_v == 2, f"kernel hardcodes stride=2, got {stride_v}"

    xs = np.ascontiguousarray(x.reshape(C * K, V))
    # (8, 8192, 1024): per-core column slice of the adjacency
    g_sh = np.ascontiguousarray(graph.reshape(V, NCORES, WS).transpose(1, 0, 2))
    # odd fifo frames 1,3,...,13 -> (8, 28, 1024) per-core slices
    ff_sh = np.ascontiguousarray(
        fifo.reshape(F, C, NCORES, WS)[1:14:2]
        .transpose(2, 0, 1, 3)
        .reshape(NCORES, 7 * C, WS)
    )
    eye = np.eye(K, dtype=np.float32)
    selr = np.ascontiguousarray(np.tile(eye, (C, 1)))
    selfm = np.ascontiguousarray(np.tile(eye, (7, 1)))

    if _CACHED_NC is None:
        _CACHED_NC = _build_nc()
    nc = _CACHED_NC

    in_maps = [
        {"g": g_sh[m], "xs": xs, "ff": ff_sh[m], "selr": selr, "selfm": selfm}
        for m in range(NCORES)
    ]
    res = run_bass_kernel_spmd(
        nc, in_maps, core_ids=list(range(NCORES)), trace=TRACE
    )
    LAST = res
    b = np.concatenate([res.results[m]["out"] for m in range(NCORES)], axis=1)
    return np.ascontiguousarray(b.reshape(1, C, V, 1))
